# revision 35
# baseline (speedup 1.0000x reference)
"""Trainium2 Bass kernel: fused concat-linear attention map + softmax.

reference:  scores[b,h,n] = key[b,n,:]@Wk[h,:] + query[b,0,:]@Wq[h,:] + bias[h]
            attn = softmax over n              (B=16, N=20000, D=256, H=8)

Sharding: batch dim B=16 split across 8 cores (2 batches/core), weights
replicated.  Per batch the kernel streams key (20.5 MB f32) through:

  SWDGE (GpSimd) DMA loads, f32 -> bf16 cast in-DMA, row-block layout
  "(p s) d -> p s d": partition p holds smod CONSECUTIVE key rows -> one
  contiguous 16 KB HBM descriptor per partition (near line rate; the naive
  "(s p) d" interleave yields 1 KB descriptors and ~283 GB/s)
    -> PE transpose of 128x128 bf16 blocks (d onto partitions; bf16
       weights trigger Fast Weight Load, ~2x the f32 LDWEIGHTS rate)
    -> DVE/ACT copy PSUM->SBUF casting to bf16 (alternating engines)
    -> PE matmuls vs tiny stationary WkT [128,8] bf16, COL-TILED: the 4
       chunks of each 2048-row load land in one PSUM bank at partition
       offsets 0/32/64/96, so one ScalarE exp covers 4 chunks at [128,512]
    -> ScalarE fused exp(x + (qWq+b)[h]) with accum_out partial sums
    -> per-batch 1/sum scale split across DVE and ACT halves (f32)
    -> packed f32 store on the otherwise-idle Sync HWDGE ring.

The DRAM output is in the packed (col-tiled, row-block) order; kernel()
un-permutes with a precomputed fancy index on the host (pure layout glue,
same class as the existing shard-gather/reshape).

Softmax without max-subtraction: scores are O(+-7) so f32 exp is safe and
mathematically identical.
"""

import sys

import numpy as np

for _p in ("/opt/trn_rl_repo",):
    if _p not in sys.path:
        sys.path.append(_p)

from contextlib import ExitStack

import concourse.bass as bass
import concourse.bacc as bacc
import concourse.tile as tile
from concourse import mybir
from concourse.masks import make_identity

B, N, D, H = 16, 20000, 256, 8
NCORES = 8
BPC = B // NCORES  # batches per core
P = 128
CHUNK = 512  # n-columns per score chunk (= one PSUM bank of f32)
F32 = mybir.dt.float32
F32R = mybir.dt.float32r
BF16 = mybir.dt.bfloat16

# per-batch load plan: (n0, smod): partition p holds rows [n0+smod*p,
# n0+smod*(p+1)); chunk c of a load = transpose blocks s in [4c, 4c+4).
LOADS = [(L * 2048, 16) for L in range(9)] + [(18432, 12)]
NPACKS = len(LOADS)  # one packed [128,512] exp per load
REM_N0, REM_ROWS = 19968, 32
PACKED = NPACKS * CHUNK  # 5120 packed columns per head-group
RAW_COLS = 3 * PACKED + 9 * CHUNK + REM_ROWS  # 20000

CAST_LOADS = True  # SWDGE f32->bf16 loads; False = HWDGE f32 loads


def _packed_pos():
    """pos[n] = column in the packed DRAM layout holding output index n.

    Packed layout per (batch, head): [g0 packs 0..9 | g1 packs 0..9 |
    g2 packs 0..9 | g3 packs 0..8 | remainder 32], where pack L's 512
    columns are j = s'*128 + p  <->  n = n0(L) + smod*p + 4c + s', c = g.
    """
    pos = np.empty(N, np.int64)
    sp = np.arange(4)[:, None]
    pp = np.arange(P)[None, :]
    for (L, (n0, smod)) in enumerate(LOADS):
        for c in range(smod // 4):
            idx = n0 + smod * pp + 4 * c + sp
            raw = c * PACKED + L * CHUNK + sp * P + pp
            pos[idx] = raw
    pos[REM_N0:] = 3 * PACKED + 9 * CHUNK + np.arange(REM_ROWS)
    return pos


POS = _packed_pos()


def build_kernel(bpc=BPC, cast_loads=CAST_LOADS):
    nc = bacc.Bacc("TRN2", target_bir_lowering=False, debug=False)
    q_in = nc.declare_dram_parameter("q", [bpc, D], F32, isOutput=False)
    k_in = nc.declare_dram_parameter("k", [bpc, N, D], F32, isOutput=False)
    w_in = nc.declare_dram_parameter("w", [H, 2 * D], F32, isOutput=False)
    b_in = nc.declare_dram_parameter("b", [H], F32, isOutput=False)
    # bf16 output (host upcasts): halves the tail store-drain, and the
    # f32->bf16 cast rides the same SWDGE ring as the loads (emitted after
    # every load, so stores never stall the load stream).
    out = nc.declare_dram_parameter("out", [bpc, H, RAW_COLS], BF16, isOutput=True)

    ld_dtype = BF16 if cast_loads else F32

    def load_dma(out_ap, in_ap):
        if cast_loads:
            nc.gpsimd.dma_start(out=out_ap, in_=in_ap)
        else:
            nc.sync.dma_start(out=out_ap, in_=in_ap)

    def store_dma(out_ap, in_ap):
        # plain bf16 -> bf16 on the otherwise-idle Sync HWDGE ring (HWDGE
        # store descriptors drain ~2x faster than SWDGE cast-stores)
        nc.sync.dma_start(out=out_ap, in_=in_ap)

    with ExitStack() as ctx:
        tc = ctx.enter_context(tile.TileContext(nc))
        consts = ctx.enter_context(tc.tile_pool(name="consts", bufs=1))
        loads = ctx.enter_context(tc.tile_pool(name="loads", bufs=7))
        kts = ctx.enter_context(tc.tile_pool(name="kts", bufs=6))
        probp = ctx.enter_context(tc.tile_pool(name="prob", bufs=2))
        small = ctx.enter_context(tc.tile_pool(name="small", bufs=2))
        psum_kt = ctx.enter_context(tc.tile_pool(name="psum_kt", bufs=2, space="PSUM"))
        psum_sc = ctx.enter_context(tc.tile_pool(name="psum_sc", bufs=2, space="PSUM"))
        psum_mi = ctx.enter_context(tc.tile_pool(name="psum_mi", bufs=1, space="PSUM"))

        # pre-issue the first key loads before the (GpSimd-queue) identity
        # setup, so the load stream starts at t~0
        pre_lds = []
        for L in range(2):
            n0, smod = LOADS[L]
            ld = loads.tile([P, 16, D], ld_dtype, tag="load")
            load_dma(
                ld[:, :smod, :],
                k_in[0, n0:n0 + P * smod, :].rearrange("(p s) d -> p s d", p=P),
            )
            pre_lds.append(ld)

        identity = consts.tile([P, P], F32)
        make_identity(nc, identity)
        if cast_loads:
            id_t = consts.tile([P, P], BF16)
            nc.vector.tensor_copy(out=id_t[:, :], in_=identity[:, :])
        else:
            id_t = identity

        # --- constants (setup DMAs ride the ACT HWDGE ring) -----------------
        w_sb = consts.tile([H, 2 * D], F32)
        nc.scalar.dma_start(out=w_sb[:, :], in_=w_in[:, :])
        b_sb = consts.tile([H, 1], F32)
        nc.scalar.dma_start(out=b_sb[:, :], in_=b_in[:])

        # wqT[:, c, :]: WqT halves (exact f32); wkT32[:, c, 0:8]: WkT halves
        # in bf16 for the score matmuls, zero-PADDED to 32 stationary columns
        # so every partition of each col-tiled group gets WRITTEN (scores 0
        # on the 24 pad rows -> exp stays finite and deterministic; the
        # gather matrices zero them out).  Matmul cost is moving-bound, so
        # the extra 24 output partitions are free.
        wqT = consts.tile([P, 2, H], F32)
        wkT32 = consts.tile([P, 2, 32], BF16)
        nc.gpsimd.memset(wkT32[:, :, :], 0.0)
        for c in range(4):
            pt = psum_mi.tile([P, H], F32, tag="mi")
            nc.tensor.transpose(pt[:, :], w_sb[:, c * P:(c + 1) * P], identity[:H, :H])
            dst = wqT[:, c, :] if c < 2 else wkT32[:, c - 2, 0:H]
            nc.vector.tensor_copy(out=dst, in_=pt[:, :])
        z32 = consts.tile([P, 32], BF16)
        nc.gpsimd.memset(z32[:, :], 0.0)

        q_sb = consts.tile([1, bpc, D], F32)
        nc.scalar.dma_start(out=q_sb[:, :, :], in_=q_in[:, :])
        qT = consts.tile([P, bpc, 2], F32)
        for i in range(bpc):
            for c in range(2):
                pt = psum_mi.tile([P, 1], F32, tag="mi")
                nc.tensor.transpose(
                    pt[:, :], q_sb[0:1, i, c * P:(c + 1) * P], identity[:1, :1]
                )
                nc.vector.tensor_copy(out=qT[:, i, c:c + 1], in_=pt[:, :])

        # qb[:, i] = Wq @ q_i + b   (full-f32 matmul; 1-row stream, trivial)
        qb = consts.tile([H, bpc], F32)
        for i in range(bpc):
            qp = psum_mi.tile([H, 1], F32, tag="mi")
            nc.tensor.matmul(
                qp[:, :], wqT[:, 0, :], qT[:, i, 0:1], start=True, stop=False
            )
            nc.tensor.matmul(
                qp[:, :], wqT[:, 1, :], qT[:, i, 1:2], start=False, stop=True
            )
            nc.vector.tensor_add(qb[:, i:i + 1], qp[:, :], b_sb[:, :])

        # group-scatter matrices: G2[h, 32g+h] = 1 (g<4), G3s likewise (g<3)
        G2 = consts.tile([H, P], F32)
        nc.scalar.memzero(G2[:, :])
        G3s = consts.tile([H, P], F32)
        nc.scalar.memzero(G3s[:, :])
        for g in range(4):
            nc.vector.tensor_copy(out=G2[:, 32 * g:32 * g + H], in_=identity[:H, :H])
            if g < 3:
                nc.vector.tensor_copy(
                    out=G3s[:, 32 * g:32 * g + H], in_=identity[:H, :H]
                )
        G = consts.tile([P, H], F32)   # gather: G[32g+h, h] = 1
        G3 = consts.tile([P, H], F32)
        for src, dst in ((G2, G), (G3s, G3)):
            pt = psum_mi.tile([P, H], F32, tag="mi")
            nc.tensor.transpose(pt[:, :], src[:, :], identity[:H, :H])
            nc.vector.tensor_copy(out=dst[:, :], in_=pt[:, :])

        # qb broadcast to the col-tiled partition layout: qb128[32g+h] = qb[h]
        qb128 = consts.tile([P, bpc], F32)
        for i in range(bpc):
            pt = psum_mi.tile([P, 1], F32, tag="mi")
            nc.tensor.matmul(pt[:, :], G2[:, :], qb[:, i:i + 1], start=True, stop=True)
            nc.vector.tensor_copy(out=qb128[:, i:i + 1], in_=pt[:, :])

        wk0 = wkT32[:, 0, :]
        wk1 = wkT32[:, 1, :]

        # --- main loop ------------------------------------------------------
        batch_tiles = []
        for i in range(bpc):
            prob = probp.tile([P, PACKED], F32, tag="prob")
            prob8 = probp.tile([H, REM_ROWS], F32, tag="prob8")
            sums = small.tile([P, NPACKS], F32, tag="sums")
            srem = small.tile([H, 1], F32, tag="srem")
            ncopy = 0
            last_k0 = None

            def chunk_scores(scp, ld, c, nsub, sub0):
                """transpose+copy+matmul chunk c (blocks s=sub0..sub0+nsub)
                into scp[32c:32c+8, :]."""
                nonlocal ncopy
                w = nsub * P
                kt0 = psum_kt.tile([P, CHUNK], ld_dtype, tag="kt0")
                kt1 = psum_kt.tile([P, CHUNK], ld_dtype, tag="kt1")
                for s in range(nsub):
                    nc.tensor.transpose(
                        kt0[:, s * P:(s + 1) * P], ld[:, sub0 + s, 0:P],
                        id_t[:, :]
                    )
                    nc.tensor.transpose(
                        kt1[:, s * P:(s + 1) * P], ld[:, sub0 + s, P:2 * P],
                        id_t[:, :]
                    )
                nonlocal last_k0
                k0 = kts.tile([P, CHUNK], BF16, tag="k0")
                k1 = kts.tile([P, CHUNK], BF16, tag="k1")
                last_k0 = k0
                # DVE copies are cheaper than ACT's (0.46 vs 0.66us), and ACT
                # also carries the exps: give DVE 3 of every 4 copies
                nc.vector.tensor_copy(out=k0[:, :w], in_=kt0[:, :w])
                if ncopy % 2 == 0:
                    nc.scalar.copy(out=k1[:, :w], in_=kt1[:, :w])
                else:
                    nc.vector.tensor_copy(out=k1[:, :w], in_=kt1[:, :w])
                ncopy += 1
                nc.tensor.matmul(
                    scp[32 * c:32 * c + 32, :w], wk0, k0[:, :w],
                    start=True, stop=False, tile_position=(0, 32 * c),
                )
                nc.tensor.matmul(
                    scp[32 * c:32 * c + 32, :w], wk1, k1[:, :w],
                    start=False, stop=True, tile_position=(0, 32 * c),
                )

            # 32-row remainder: load + process FIRST, off the critical tail
            rem_ld = loads.tile([REM_ROWS, D], ld_dtype, tag="rem_ld")
            load_dma(rem_ld[:, :], k_in[i, REM_N0:REM_N0 + REM_ROWS, :])
            scp = psum_sc.tile([P, CHUNK], F32, tag="sc")
            kt0 = psum_kt.tile([P, CHUNK], ld_dtype, tag="kt0")
            kt1 = psum_kt.tile([P, CHUNK], ld_dtype, tag="kt1")
            nc.tensor.transpose(
                kt0[:, :REM_ROWS], rem_ld[:, 0:P], id_t[:REM_ROWS, :REM_ROWS]
            )
            nc.tensor.transpose(
                kt1[:, :REM_ROWS], rem_ld[:, P:2 * P], id_t[:REM_ROWS, :REM_ROWS]
            )
            k0 = kts.tile([P, CHUNK], BF16, tag="k0")
            k1 = kts.tile([P, CHUNK], BF16, tag="k1")
            nc.vector.tensor_copy(out=k0[:, :REM_ROWS], in_=kt0[:, :REM_ROWS])
            nc.scalar.copy(out=k1[:, :REM_ROWS], in_=kt1[:, :REM_ROWS])
            nc.tensor.matmul(
                scp[:H, :REM_ROWS], wk0[:, :H], k0[:, :REM_ROWS],
                start=True, stop=False,
            )
            nc.tensor.matmul(
                scp[:H, :REM_ROWS], wk1[:, :H], k1[:, :REM_ROWS],
                start=False, stop=True,
            )
            nc.scalar.activation(
                out=prob8[:, :],
                in_=scp[:H, :REM_ROWS],
                func=mybir.ActivationFunctionType.Exp,
                bias=qb[:, i:i + 1],
                scale=1.0,
                accum_out=srem[:, :],
            )

            for (L, (n0, smod)) in enumerate(LOADS):
                rows = P * smod
                if i == 0 and L < len(pre_lds):
                    ld = pre_lds[L]
                else:
                    ld = loads.tile([P, 16, D], ld_dtype, tag="load")
                    # partition p <- rows [n0+smod*p, n0+smod*(p+1)): one
                    # contiguous smod-KB descriptor per partition.
                    load_dma(
                        ld[:, :smod, :],
                        k_in[i, n0:n0 + rows, :].rearrange("(p s) d -> p s d", p=P),
                    )
                scp = psum_sc.tile([P, CHUNK], F32, tag="sc")
                for c in range(smod // 4):
                    chunk_scores(scp, ld, c, 4, 4 * c)
                if smod == 12:
                    # tail pack has no group 3: write real zeros there so the
                    # exp input is deterministic and finite
                    nc.tensor.matmul(
                        scp[96:128, :], z32, last_k0[:, :],
                        start=True, stop=True, tile_position=(0, 96),
                    )
                nc.scalar.activation(
                    out=prob[:, L * CHUNK:(L + 1) * CHUNK],
                    in_=scp[:, :],
                    func=mybir.ActivationFunctionType.Exp,
                    bias=qb128[:, i:i + 1],
                    scale=1.0,
                    accum_out=sums[:, L:L + 1],
                )
            # total = sum over groups of pack sums (G3 excludes the tail
            # pack's unused group 3) + remainder
            rmain = small.tile([P, 1], F32, tag="rmain")
            nc.vector.reduce_sum(
                out=rmain[:, :], in_=sums[:, :NPACKS - 1], axis=mybir.AxisListType.X
            )
            pt8 = psum_mi.tile([H, 1], F32, tag="mi")
            nc.tensor.matmul(pt8[:, :], G[:, :], rmain[:, :], start=True, stop=False)
            nc.tensor.matmul(
                pt8[:, :], G3[:, :], sums[:, NPACKS - 1:NPACKS],
                start=False, stop=True,
            )
            tot = small.tile([H, 1], F32, tag="tot")
            nc.vector.tensor_add(tot[:, :], pt8[:, :], srem[:, :])
            rec = small.tile([H, 1], F32, tag="rec")
            nc.vector.reciprocal(out=rec[:, :], in_=tot[:, :])
            rec128 = small.tile([P, 1], F32, tag="rec128")
            ptr = psum_mi.tile([P, 1], F32, tag="mi")
            nc.tensor.matmul(ptr[:, :], G2[:, :], rec[:, :], start=True, stop=True)
            nc.vector.tensor_copy(out=rec128[:, :], in_=ptr[:, :])

            # scale writes a bf16 copy (the store dtype) so stores need no
            # in-DMA cast; 4 column pieces alternate DVE/ACT for latency
            prob_bf = probp.tile([P, PACKED], BF16, tag="prob_bf")
            prob8_bf = probp.tile([H, REM_ROWS], BF16, tag="prob8_bf")
            qp = PACKED // 4
            for pc in range(4):
                sl = slice(pc * qp, (pc + 1) * qp)
                if pc % 2 == 0:
                    nc.vector.tensor_scalar_mul(
                        prob_bf[:, sl], prob[:, sl], rec128[:, :]
                    )
                else:
                    nc.scalar.mul(prob_bf[:, sl], prob[:, sl], rec128[:, :])
            nc.scalar.mul(prob8_bf[:, :], prob8[:, :], rec[:, :])
            batch_tiles.append((prob_bf, prob8_bf))

        # packed stores, DEFERRED until after every load is emitted: they
        # share the GpSimd SWDGE queue with the loads (f32 -> bf16 cast),
        # so batch 0's stores must queue behind batch 1's loads to avoid
        # stalling the load stream.
        for i, (prob, prob8) in enumerate(batch_tiles):
            for g in range(4):
                width = PACKED if g < 3 else PACKED - CHUNK
                off = g * PACKED
                store_dma(
                    out[i, :, off:off + width],
                    prob[32 * g:32 * g + H, :width],
                )
            store_dma(out[i, :, RAW_COLS - REM_ROWS:], prob8[:, :])

    nc.compile()
    return nc


_NC_CACHE = {}


def _get_nc():
    if "nc" not in _NC_CACHE:
        _NC_CACHE["nc"] = build_kernel()
    return _NC_CACHE["nc"]


def unpermute(raw):
    """raw [B, H, RAW_COLS] packed bf16 layout -> [B, H, N] f32 natural."""
    return np.ascontiguousarray(np.asarray(raw).astype(np.float32)[:, :, POS])


def kernel(query, key, W, b):
    from concourse.bass_utils import run_bass_kernel_spmd

    query = np.ascontiguousarray(np.asarray(query, np.float32).reshape(B, D))
    key = np.ascontiguousarray(np.asarray(key, np.float32))
    W = np.ascontiguousarray(np.asarray(W, np.float32))
    b = np.ascontiguousarray(np.asarray(b, np.float32))

    nc = _get_nc()
    in_maps = []
    for c in range(NCORES):
        s = slice(BPC * c, BPC * (c + 1))
        in_maps.append(
            {
                "q": query[s],
                "k": key[s],
                "w": W,
                "b": b,
            }
        )
    res = run_bass_kernel_spmd(nc, in_maps, list(range(NCORES))).results
    raw = np.concatenate([res[c]["out"] for c in range(NCORES)], axis=0)
    return unpermute(raw)


# revision 36
# speedup vs baseline: 1.0238x; 1.0238x over previous
"""Trainium2 Bass kernel: fused concat-linear attention map + softmax.

reference:  scores[b,h,n] = key[b,n,:]@Wk[h,:] + query[b,0,:]@Wq[h,:] + bias[h]
            attn = softmax over n              (B=16, N=20000, D=256, H=8)

Sharding: batch dim B=16 split across 8 cores (2 batches/core), weights
replicated.  Per batch the kernel streams key (20.5 MB f32) through:

  SWDGE (GpSimd) DMA loads, f32 -> bf16 cast in-DMA, row-block layout
  "(p s) d -> p s d": partition p holds smod CONSECUTIVE key rows -> one
  contiguous 16 KB HBM descriptor per partition (near line rate; the naive
  "(s p) d" interleave yields 1 KB descriptors and ~283 GB/s)
    -> PE transpose of 128x128 bf16 blocks (d onto partitions; bf16
       weights trigger Fast Weight Load, ~2x the f32 LDWEIGHTS rate)
    -> DVE/ACT copy PSUM->SBUF casting to bf16 (alternating engines)
    -> PE matmuls vs tiny stationary WkT [128,8] bf16, COL-TILED: the 4
       chunks of each 2048-row load land in one PSUM bank at partition
       offsets 0/32/64/96, so one ScalarE exp covers 4 chunks at [128,512]
    -> ScalarE fused exp(x + (qWq+b)[h]) with accum_out partial sums
    -> per-batch 1/sum scale split across DVE and ACT halves (f32)
    -> packed f32 store on the otherwise-idle Sync HWDGE ring.

The DRAM output is in the packed (col-tiled, row-block) order; kernel()
un-permutes with a precomputed fancy index on the host (pure layout glue,
same class as the existing shard-gather/reshape).

Softmax without max-subtraction: scores are O(+-7) so f32 exp is safe and
mathematically identical.
"""

import sys

import numpy as np

for _p in ("/opt/trn_rl_repo",):
    if _p not in sys.path:
        sys.path.append(_p)

from contextlib import ExitStack

import concourse.bass as bass
import concourse.bacc as bacc
import concourse.tile as tile
from concourse import mybir
from concourse.masks import make_identity

B, N, D, H = 16, 20000, 256, 8
NCORES = 8
BPC = B // NCORES  # batches per core
P = 128
CHUNK = 512  # n-columns per score chunk (= one PSUM bank of f32)
F32 = mybir.dt.float32
F32R = mybir.dt.float32r
BF16 = mybir.dt.bfloat16

# per-batch load plan: (n0, smod): partition p holds rows [n0+smod*p,
# n0+smod*(p+1)); chunk c of a load = transpose blocks s in [4c, 4c+4).
LOADS = [(L * 2048, 16) for L in range(9)] + [(18432, 12)]
NPACKS = len(LOADS)  # one packed [128,512] exp per load
REM_N0, REM_ROWS = 19968, 32
PACKED = NPACKS * CHUNK  # 5120 packed columns per head-group
RAW_COLS = 3 * PACKED + 9 * CHUNK + REM_ROWS  # 20000

CAST_LOADS = True  # SWDGE f32->bf16 loads; False = HWDGE f32 loads


def _packed_pos():
    """pos[n] = column in the packed DRAM layout holding output index n.

    Packed layout per (batch, head): [g0 packs 0..9 | g1 packs 0..9 |
    g2 packs 0..9 | g3 packs 0..8 | remainder 32], where pack L's 512
    columns are j = s'*128 + p  <->  n = n0(L) + smod*p + 4c + s', c = g.
    """
    pos = np.empty(N, np.int64)
    sp = np.arange(4)[:, None]
    pp = np.arange(P)[None, :]
    for (L, (n0, smod)) in enumerate(LOADS):
        for c in range(smod // 4):
            idx = n0 + smod * pp + 4 * c + sp
            raw = c * PACKED + L * CHUNK + sp * P + pp
            pos[idx] = raw
    pos[REM_N0:] = 3 * PACKED + 9 * CHUNK + np.arange(REM_ROWS)
    return pos


POS = _packed_pos()


def build_kernel(bpc=BPC, cast_loads=CAST_LOADS):
    nc = bacc.Bacc("TRN2", target_bir_lowering=False, debug=False)
    q_in = nc.declare_dram_parameter("q", [bpc, D], F32, isOutput=False)
    k_in = nc.declare_dram_parameter("k", [bpc, N, D], F32, isOutput=False)
    w_in = nc.declare_dram_parameter("w", [H, 2 * D], F32, isOutput=False)
    b_in = nc.declare_dram_parameter("b", [H], F32, isOutput=False)
    # bf16 output (host upcasts): halves the tail store-drain, and the
    # f32->bf16 cast rides the same SWDGE ring as the loads (emitted after
    # every load, so stores never stall the load stream).
    out = nc.declare_dram_parameter("out", [bpc, H, RAW_COLS], BF16, isOutput=True)

    ld_dtype = BF16 if cast_loads else F32

    def load_dma(out_ap, in_ap):
        if cast_loads:
            nc.gpsimd.dma_start(out=out_ap, in_=in_ap)
        else:
            nc.sync.dma_start(out=out_ap, in_=in_ap)

    def store_dma(out_ap, in_ap):
        # plain bf16 -> bf16 on the otherwise-idle Sync HWDGE ring (HWDGE
        # store descriptors drain ~2x faster than SWDGE cast-stores)
        nc.sync.dma_start(out=out_ap, in_=in_ap)

    with ExitStack() as ctx:
        tc = ctx.enter_context(tile.TileContext(nc))
        consts = ctx.enter_context(tc.tile_pool(name="consts", bufs=1))
        loads = ctx.enter_context(tc.tile_pool(name="loads", bufs=5))
        kts = ctx.enter_context(tc.tile_pool(name="kts", bufs=4))
        probp = ctx.enter_context(tc.tile_pool(name="prob", bufs=2))
        small = ctx.enter_context(tc.tile_pool(name="small", bufs=2))
        psum_kt = ctx.enter_context(tc.tile_pool(name="psum_kt", bufs=2, space="PSUM"))
        psum_sc = ctx.enter_context(tc.tile_pool(name="psum_sc", bufs=2, space="PSUM"))
        psum_mi = ctx.enter_context(tc.tile_pool(name="psum_mi", bufs=1, space="PSUM"))

        # pre-issue the first key loads before the (GpSimd-queue) identity
        # setup, so the load stream starts at t~0
        pre_lds = []
        for L in range(2):
            n0, smod = LOADS[L]
            ld = loads.tile([P, 16, D], ld_dtype, tag="load")
            load_dma(
                ld[:, :smod, :],
                k_in[0, n0:n0 + P * smod, :].rearrange("(p s) d -> p s d", p=P),
            )
            pre_lds.append(ld)

        identity = consts.tile([P, P], F32)
        make_identity(nc, identity)
        if cast_loads:
            id_t = consts.tile([P, P], BF16)
            nc.vector.tensor_copy(out=id_t[:, :], in_=identity[:, :])
        else:
            id_t = identity

        # --- constants (setup DMAs ride the ACT HWDGE ring) -----------------
        w_sb = consts.tile([H, 2 * D], F32)
        nc.scalar.dma_start(out=w_sb[:, :], in_=w_in[:, :])
        b_sb = consts.tile([H, 1], F32)
        nc.scalar.dma_start(out=b_sb[:, :], in_=b_in[:])

        # wqT[:, c, :]: WqT halves (exact f32); wkT32[:, c, 0:8]: WkT halves
        # in bf16 for the score matmuls, zero-PADDED to 32 stationary columns
        # so every partition of each col-tiled group gets WRITTEN (scores 0
        # on the 24 pad rows -> exp stays finite and deterministic; the
        # gather matrices zero them out).  Matmul cost is moving-bound, so
        # the extra 24 output partitions are free.
        wqT = consts.tile([P, 2, H], F32)
        wkT32 = consts.tile([P, 2, 32], BF16)
        nc.gpsimd.memset(wkT32[:, :, :], 0.0)
        for c in range(4):
            pt = psum_mi.tile([P, H], F32, tag="mi")
            nc.tensor.transpose(pt[:, :], w_sb[:, c * P:(c + 1) * P], identity[:H, :H])
            dst = wqT[:, c, :] if c < 2 else wkT32[:, c - 2, 0:H]
            nc.vector.tensor_copy(out=dst, in_=pt[:, :])
        z32 = consts.tile([P, 32], BF16)
        nc.gpsimd.memset(z32[:, :], 0.0)

        q_sb = consts.tile([1, bpc, D], F32)
        nc.scalar.dma_start(out=q_sb[:, :, :], in_=q_in[:, :])
        qT = consts.tile([P, bpc, 2], F32)
        for i in range(bpc):
            for c in range(2):
                pt = psum_mi.tile([P, 1], F32, tag="mi")
                nc.tensor.transpose(
                    pt[:, :], q_sb[0:1, i, c * P:(c + 1) * P], identity[:1, :1]
                )
                nc.vector.tensor_copy(out=qT[:, i, c:c + 1], in_=pt[:, :])

        # qb[:, i] = Wq @ q_i + b   (full-f32 matmul; 1-row stream, trivial)
        qb = consts.tile([H, bpc], F32)
        for i in range(bpc):
            qp = psum_mi.tile([H, 1], F32, tag="mi")
            nc.tensor.matmul(
                qp[:, :], wqT[:, 0, :], qT[:, i, 0:1], start=True, stop=False
            )
            nc.tensor.matmul(
                qp[:, :], wqT[:, 1, :], qT[:, i, 1:2], start=False, stop=True
            )
            nc.vector.tensor_add(qb[:, i:i + 1], qp[:, :], b_sb[:, :])

        # group-scatter matrices: G2[h, 32g+h] = 1 (g<4), G3s likewise (g<3)
        G2 = consts.tile([H, P], F32)
        nc.scalar.memzero(G2[:, :])
        G3s = consts.tile([H, P], F32)
        nc.scalar.memzero(G3s[:, :])
        for g in range(4):
            nc.vector.tensor_copy(out=G2[:, 32 * g:32 * g + H], in_=identity[:H, :H])
            if g < 3:
                nc.vector.tensor_copy(
                    out=G3s[:, 32 * g:32 * g + H], in_=identity[:H, :H]
                )
        G = consts.tile([P, H], F32)   # gather: G[32g+h, h] = 1
        G3 = consts.tile([P, H], F32)
        for src, dst in ((G2, G), (G3s, G3)):
            pt = psum_mi.tile([P, H], F32, tag="mi")
            nc.tensor.transpose(pt[:, :], src[:, :], identity[:H, :H])
            nc.vector.tensor_copy(out=dst[:, :], in_=pt[:, :])

        # qb broadcast to the col-tiled partition layout: qb128[32g+h] = qb[h]
        qb128 = consts.tile([P, bpc], F32)
        for i in range(bpc):
            pt = psum_mi.tile([P, 1], F32, tag="mi")
            nc.tensor.matmul(pt[:, :], G2[:, :], qb[:, i:i + 1], start=True, stop=True)
            nc.vector.tensor_copy(out=qb128[:, i:i + 1], in_=pt[:, :])

        wk0 = wkT32[:, 0, :]
        wk1 = wkT32[:, 1, :]

        # --- main loop ------------------------------------------------------
        batch_tiles = []
        for i in range(bpc):
            prob = probp.tile([P, PACKED], F32, tag="prob")
            prob8 = probp.tile([H, REM_ROWS], F32, tag="prob8")
            sums = small.tile([P, NPACKS], F32, tag="sums")
            srem = small.tile([H, 1], F32, tag="srem")
            ncopy = 0
            last_k0 = None

            def chunk_scores(scp, ld, c, nsub, sub0):
                """transpose+copy+matmul chunk c (blocks s=sub0..sub0+nsub)
                into scp[32c:32c+8, :]."""
                nonlocal ncopy
                w = nsub * P
                kt0 = psum_kt.tile([P, CHUNK], ld_dtype, tag="kt0")
                kt1 = psum_kt.tile([P, CHUNK], ld_dtype, tag="kt1")
                for s in range(nsub):
                    nc.tensor.transpose(
                        kt0[:, s * P:(s + 1) * P], ld[:, sub0 + s, 0:P],
                        id_t[:, :]
                    )
                    nc.tensor.transpose(
                        kt1[:, s * P:(s + 1) * P], ld[:, sub0 + s, P:2 * P],
                        id_t[:, :]
                    )
                nonlocal last_k0
                k0 = kts.tile([P, CHUNK], BF16, tag="k0")
                k1 = kts.tile([P, CHUNK], BF16, tag="k1")
                last_k0 = k0
                # DVE copies are cheaper than ACT's (0.46 vs 0.66us), and ACT
                # also carries the exps: give DVE 3 of every 4 copies
                nc.vector.tensor_copy(out=k0[:, :w], in_=kt0[:, :w])
                if ncopy % 2 == 0:
                    nc.scalar.copy(out=k1[:, :w], in_=kt1[:, :w])
                else:
                    nc.vector.tensor_copy(out=k1[:, :w], in_=kt1[:, :w])
                ncopy += 1
                nc.tensor.matmul(
                    scp[32 * c:32 * c + 32, :w], wk0, k0[:, :w],
                    start=True, stop=False, tile_position=(0, 32 * c),
                )
                nc.tensor.matmul(
                    scp[32 * c:32 * c + 32, :w], wk1, k1[:, :w],
                    start=False, stop=True, tile_position=(0, 32 * c),
                )

            # 32-row remainder: load + process FIRST, off the critical tail
            rem_ld = loads.tile([REM_ROWS, D], ld_dtype, tag="rem_ld")
            load_dma(rem_ld[:, :], k_in[i, REM_N0:REM_N0 + REM_ROWS, :])
            scp = psum_sc.tile([P, CHUNK], F32, tag="sc")
            kt0 = psum_kt.tile([P, CHUNK], ld_dtype, tag="kt0")
            kt1 = psum_kt.tile([P, CHUNK], ld_dtype, tag="kt1")
            nc.tensor.transpose(
                kt0[:, :REM_ROWS], rem_ld[:, 0:P], id_t[:REM_ROWS, :REM_ROWS]
            )
            nc.tensor.transpose(
                kt1[:, :REM_ROWS], rem_ld[:, P:2 * P], id_t[:REM_ROWS, :REM_ROWS]
            )
            k0 = kts.tile([P, CHUNK], BF16, tag="k0")
            k1 = kts.tile([P, CHUNK], BF16, tag="k1")
            nc.vector.tensor_copy(out=k0[:, :REM_ROWS], in_=kt0[:, :REM_ROWS])
            nc.scalar.copy(out=k1[:, :REM_ROWS], in_=kt1[:, :REM_ROWS])
            nc.tensor.matmul(
                scp[:H, :REM_ROWS], wk0[:, :H], k0[:, :REM_ROWS],
                start=True, stop=False,
            )
            nc.tensor.matmul(
                scp[:H, :REM_ROWS], wk1[:, :H], k1[:, :REM_ROWS],
                start=False, stop=True,
            )
            nc.scalar.activation(
                out=prob8[:, :],
                in_=scp[:H, :REM_ROWS],
                func=mybir.ActivationFunctionType.Exp,
                bias=qb[:, i:i + 1],
                scale=1.0,
                accum_out=srem[:, :],
            )

            for (L, (n0, smod)) in enumerate(LOADS):
                rows = P * smod
                if i == 0 and L < len(pre_lds):
                    ld = pre_lds[L]
                else:
                    ld = loads.tile([P, 16, D], ld_dtype, tag="load")
                    # partition p <- rows [n0+smod*p, n0+smod*(p+1)): one
                    # contiguous smod-KB descriptor per partition.
                    load_dma(
                        ld[:, :smod, :],
                        k_in[i, n0:n0 + rows, :].rearrange("(p s) d -> p s d", p=P),
                    )
                scp = psum_sc.tile([P, CHUNK], F32, tag="sc")
                for c in range(smod // 4):
                    chunk_scores(scp, ld, c, 4, 4 * c)
                if smod == 12:
                    # tail pack has no group 3: write real zeros there so the
                    # exp input is deterministic and finite
                    nc.tensor.matmul(
                        scp[96:128, :], z32, last_k0[:, :],
                        start=True, stop=True, tile_position=(0, 96),
                    )
                nc.scalar.activation(
                    out=prob[:, L * CHUNK:(L + 1) * CHUNK],
                    in_=scp[:, :],
                    func=mybir.ActivationFunctionType.Exp,
                    bias=qb128[:, i:i + 1],
                    scale=1.0,
                    accum_out=sums[:, L:L + 1],
                )
            # total = sum over groups of pack sums (G3 excludes the tail
            # pack's unused group 3) + remainder
            rmain = small.tile([P, 1], F32, tag="rmain")
            nc.vector.reduce_sum(
                out=rmain[:, :], in_=sums[:, :NPACKS - 1], axis=mybir.AxisListType.X
            )
            pt8 = psum_mi.tile([H, 1], F32, tag="mi")
            nc.tensor.matmul(pt8[:, :], G[:, :], rmain[:, :], start=True, stop=False)
            nc.tensor.matmul(
                pt8[:, :], G3[:, :], sums[:, NPACKS - 1:NPACKS],
                start=False, stop=True,
            )
            tot = small.tile([H, 1], F32, tag="tot")
            nc.vector.tensor_add(tot[:, :], pt8[:, :], srem[:, :])
            rec = small.tile([H, 1], F32, tag="rec")
            nc.vector.reciprocal(out=rec[:, :], in_=tot[:, :])
            rec128 = small.tile([P, 1], F32, tag="rec128")
            ptr = psum_mi.tile([P, 1], F32, tag="mi")
            nc.tensor.matmul(ptr[:, :], G2[:, :], rec[:, :], start=True, stop=True)
            nc.vector.tensor_copy(out=rec128[:, :], in_=ptr[:, :])

            # scale writes a bf16 copy (the store dtype) so stores need no
            # in-DMA cast; 4 column pieces alternate DVE/ACT for latency
            prob_bf = probp.tile([P, PACKED], BF16, tag="prob_bf")
            prob8_bf = probp.tile([H, REM_ROWS], BF16, tag="prob8_bf")
            qp = PACKED // 4
            for pc in range(4):
                sl = slice(pc * qp, (pc + 1) * qp)
                if pc % 2 == 0:
                    nc.vector.tensor_scalar_mul(
                        prob_bf[:, sl], prob[:, sl], rec128[:, :]
                    )
                else:
                    nc.scalar.mul(prob_bf[:, sl], prob[:, sl], rec128[:, :])
            nc.scalar.mul(prob8_bf[:, :], prob8[:, :], rec[:, :])
            batch_tiles.append((prob_bf, prob8_bf))

        # packed stores, DEFERRED until after every load is emitted: they
        # share the GpSimd SWDGE queue with the loads (f32 -> bf16 cast),
        # so batch 0's stores must queue behind batch 1's loads to avoid
        # stalling the load stream.
        for i, (prob, prob8) in enumerate(batch_tiles):
            for g in range(4):
                width = PACKED if g < 3 else PACKED - CHUNK
                off = g * PACKED
                store_dma(
                    out[i, :, off:off + width],
                    prob[32 * g:32 * g + H, :width],
                )
            store_dma(out[i, :, RAW_COLS - REM_ROWS:], prob8[:, :])

    nc.compile()
    return nc


_NC_CACHE = {}


def _get_nc():
    if "nc" not in _NC_CACHE:
        _NC_CACHE["nc"] = build_kernel()
    return _NC_CACHE["nc"]


def unpermute(raw):
    """raw [B, H, RAW_COLS] packed bf16 layout -> [B, H, N] f32 natural."""
    return np.ascontiguousarray(np.asarray(raw).astype(np.float32)[:, :, POS])


def kernel(query, key, W, b):
    from concourse.bass_utils import run_bass_kernel_spmd

    query = np.ascontiguousarray(np.asarray(query, np.float32).reshape(B, D))
    key = np.ascontiguousarray(np.asarray(key, np.float32))
    W = np.ascontiguousarray(np.asarray(W, np.float32))
    b = np.ascontiguousarray(np.asarray(b, np.float32))

    nc = _get_nc()
    in_maps = []
    for c in range(NCORES):
        s = slice(BPC * c, BPC * (c + 1))
        in_maps.append(
            {
                "q": query[s],
                "k": key[s],
                "w": W,
                "b": b,
            }
        )
    res = run_bass_kernel_spmd(nc, in_maps, list(range(NCORES))).results
    raw = np.concatenate([res[c]["out"] for c in range(NCORES)], axis=0)
    return unpermute(raw)


# revision 42
# speedup vs baseline: 1.1668x; 1.1397x over previous
"""Trainium2 Bass kernel: fused concat-linear attention map + softmax.

reference:  scores[b,h,n] = key[b,n,:]@Wk[h,:] + query[b,0,:]@Wq[h,:] + bias[h]
            attn = softmax over n              (B=16, N=20000, D=256, H=8)

Sharding: batch dim B=16 split across 8 cores (2 batches/core), weights
replicated.  Per batch the kernel streams key (20.5 MB f32) through:

  SWDGE (GpSimd) DMA loads, f32 -> bf16 cast in-DMA, row-block layout
  "(p s) d -> p s d": partition p holds smod CONSECUTIVE key rows -> one
  contiguous 16 KB HBM descriptor per partition (near line rate; the naive
  "(s p) d" interleave yields 1 KB descriptors and ~283 GB/s)
    -> PE transpose of 128x128 bf16 blocks (d onto partitions; bf16
       weights trigger Fast Weight Load, ~2x the f32 LDWEIGHTS rate)
    -> DVE/ACT copy PSUM->SBUF casting to bf16 (alternating engines)
    -> PE matmuls vs tiny stationary WkT [128,8] bf16, COL-TILED: the 4
       chunks of each 2048-row load land in one PSUM bank at partition
       offsets 0/32/64/96, so one ScalarE exp covers 4 chunks at [128,512]
    -> ScalarE fused exp(x + (qWq+b)[h]) with accum_out partial sums
    -> per-batch 1/sum scale split across DVE and ACT halves (f32)
    -> packed f32 store on the otherwise-idle Sync HWDGE ring.

The DRAM output is in the packed (col-tiled, row-block) order; kernel()
un-permutes with a precomputed fancy index on the host (pure layout glue,
same class as the existing shard-gather/reshape).

Softmax without max-subtraction: scores are O(+-7) so f32 exp is safe and
mathematically identical.
"""

import sys

import numpy as np

for _p in ("/opt/trn_rl_repo",):
    if _p not in sys.path:
        sys.path.append(_p)

from contextlib import ExitStack

import concourse.bass as bass
import concourse.bacc as bacc
import concourse.tile as tile
from concourse import mybir
from concourse.masks import make_identity

B, N, D, H = 16, 20000, 256, 8
NCORES = 8
BPC = B // NCORES  # batches per core
P = 128
CHUNK = 512  # n-columns per score chunk (= one PSUM bank of f32)
F32 = mybir.dt.float32
F32R = mybir.dt.float32r
BF16 = mybir.dt.bfloat16

# per-batch load plan: (n0, smod): partition p holds rows [n0+smod*p,
# n0+smod*(p+1)); chunk c of a load = transpose blocks s in [4c, 4c+4).
LOADS = [(L * 2048, 16) for L in range(9)] + [(18432, 12)]
NPACKS = len(LOADS)  # one packed [128,512] exp per load
REM_N0, REM_ROWS = 19968, 32
PACKED = NPACKS * CHUNK  # 5120 packed columns per head-group
RAW_COLS = 3 * PACKED + 9 * CHUNK + REM_ROWS  # 20000

CAST_LOADS = True  # SWDGE f32->bf16 loads; False = HWDGE f32 loads


def _packed_pos():
    """pos[n] = column in the packed DRAM layout holding output index n.

    Packed layout per (batch, head): [g0 packs 0..9 | g1 packs 0..9 |
    g2 packs 0..9 | g3 packs 0..8 | remainder 32], where pack L's 512
    columns are j = s'*128 + p  <->  n = n0(L) + smod*p + 4c + s', c = g.
    """
    pos = np.empty(N, np.int64)
    sp = np.arange(4)[:, None]
    pp = np.arange(P)[None, :]
    for (L, (n0, smod)) in enumerate(LOADS):
        for c in range(smod // 4):
            idx = n0 + smod * pp + 4 * c + sp
            raw = c * PACKED + L * CHUNK + sp * P + pp
            pos[idx] = raw
    pos[REM_N0:] = 3 * PACKED + 9 * CHUNK + np.arange(REM_ROWS)
    return pos


POS = _packed_pos()


def build_kernel(bpc=BPC, cast_loads=CAST_LOADS):
    nc = bacc.Bacc("TRN2", target_bir_lowering=False, debug=False)
    q_in = nc.declare_dram_parameter("q", [bpc, D], F32, isOutput=False)
    k_in = nc.declare_dram_parameter("k", [bpc, N, D], F32, isOutput=False)
    w_in = nc.declare_dram_parameter("w", [H, 2 * D], F32, isOutput=False)
    b_in = nc.declare_dram_parameter("b", [H], F32, isOutput=False)
    # bf16 output (host upcasts): halves the tail store-drain, and the
    # f32->bf16 cast rides the same SWDGE ring as the loads (emitted after
    # every load, so stores never stall the load stream).
    out = nc.declare_dram_parameter("out", [bpc, H, RAW_COLS], BF16, isOutput=True)

    ld_dtype = BF16 if cast_loads else F32

    def load_dma(out_ap, in_ap):
        if cast_loads:
            nc.gpsimd.dma_start(out=out_ap, in_=in_ap)
        else:
            nc.sync.dma_start(out=out_ap, in_=in_ap)

    _store_ring = [0]

    def store_dma(out_ap, in_ap):
        # plain bf16 -> bf16, alternating between the Sync and Scalar HWDGE
        # rings: the store partitions (32g+h) map to only 4 SDMA engines, so
        # two rings double the per-engine descriptor concurrency at the tail
        eng = nc.sync if _store_ring[0] % 2 == 0 else nc.scalar
        _store_ring[0] += 1
        eng.dma_start(out=out_ap, in_=in_ap)

    with ExitStack() as ctx:
        tc = ctx.enter_context(tile.TileContext(nc))
        consts = ctx.enter_context(tc.tile_pool(name="consts", bufs=1))
        loads = ctx.enter_context(tc.tile_pool(name="loads", bufs=5))
        kts = ctx.enter_context(tc.tile_pool(name="kts", bufs=4))
        probp = ctx.enter_context(tc.tile_pool(name="prob", bufs=2))
        small = ctx.enter_context(tc.tile_pool(name="small", bufs=2))
        psum_kt = ctx.enter_context(tc.tile_pool(name="psum_kt", bufs=4, space="PSUM"))
        psum_sc = ctx.enter_context(tc.tile_pool(name="psum_sc", bufs=2, space="PSUM"))
        psum_mi = ctx.enter_context(tc.tile_pool(name="psum_mi", bufs=1, space="PSUM"))

        # pre-issue the first key loads before the (GpSimd-queue) identity
        # setup, so the load stream starts at t~0
        pre_lds = []
        for L in range(2):
            n0, smod = LOADS[L]
            ld = loads.tile([P, 16, D], ld_dtype, tag="load")
            load_dma(
                ld[:, :smod, :],
                k_in[0, n0:n0 + P * smod, :].rearrange("(p s) d -> p s d", p=P),
            )
            pre_lds.append(ld)

        identity = consts.tile([P, P], F32)
        make_identity(nc, identity)
        if cast_loads:
            id_t = consts.tile([P, P], BF16)
            nc.vector.tensor_copy(out=id_t[:, :], in_=identity[:, :])
        else:
            id_t = identity

        # --- constants (setup DMAs ride the ACT HWDGE ring) -----------------
        w_sb = consts.tile([H, 2 * D], F32)
        nc.scalar.dma_start(out=w_sb[:, :], in_=w_in[:, :])
        b_sb = consts.tile([H, 1], F32)
        nc.scalar.dma_start(out=b_sb[:, :], in_=b_in[:])

        # wqT[:, c, :]: WqT halves (exact f32); wkT32[:, c, 0:8]: WkT halves
        # in bf16 for the score matmuls, zero-PADDED to 32 stationary columns
        # so every partition of each col-tiled group gets WRITTEN (scores 0
        # on the 24 pad rows -> exp stays finite and deterministic; the
        # gather matrices zero them out).  Matmul cost is moving-bound, so
        # the extra 24 output partitions are free.
        wqT = consts.tile([P, 2, H], F32)
        wkT32 = consts.tile([P, 2, 32], BF16)
        nc.gpsimd.memset(wkT32[:, :, :], 0.0)
        for c in range(4):
            pt = psum_mi.tile([P, H], F32, tag="mi")
            nc.tensor.transpose(pt[:, :], w_sb[:, c * P:(c + 1) * P], identity[:H, :H])
            dst = wqT[:, c, :] if c < 2 else wkT32[:, c - 2, 0:H]
            nc.vector.tensor_copy(out=dst, in_=pt[:, :])
        z32 = consts.tile([P, 32], BF16)
        nc.gpsimd.memset(z32[:, :], 0.0)

        q_sb = consts.tile([1, bpc, D], F32)
        nc.scalar.dma_start(out=q_sb[:, :, :], in_=q_in[:, :])
        qT = consts.tile([P, bpc, 2], F32)
        for i in range(bpc):
            for c in range(2):
                pt = psum_mi.tile([P, 1], F32, tag="mi")
                nc.tensor.transpose(
                    pt[:, :], q_sb[0:1, i, c * P:(c + 1) * P], identity[:1, :1]
                )
                nc.vector.tensor_copy(out=qT[:, i, c:c + 1], in_=pt[:, :])

        # qb[:, i] = Wq @ q_i + b   (full-f32 matmul; 1-row stream, trivial)
        qb = consts.tile([H, bpc], F32)
        for i in range(bpc):
            qp = psum_mi.tile([H, 1], F32, tag="mi")
            nc.tensor.matmul(
                qp[:, :], wqT[:, 0, :], qT[:, i, 0:1], start=True, stop=False
            )
            nc.tensor.matmul(
                qp[:, :], wqT[:, 1, :], qT[:, i, 1:2], start=False, stop=True
            )
            nc.vector.tensor_add(qb[:, i:i + 1], qp[:, :], b_sb[:, :])

        # group-scatter matrices: G2[h, 32g+h] = 1 (g<4), G3s likewise (g<3)
        G2 = consts.tile([H, P], F32)
        nc.scalar.memzero(G2[:, :])
        G3s = consts.tile([H, P], F32)
        nc.scalar.memzero(G3s[:, :])
        for g in range(4):
            nc.vector.tensor_copy(out=G2[:, 32 * g:32 * g + H], in_=identity[:H, :H])
            if g < 3:
                nc.vector.tensor_copy(
                    out=G3s[:, 32 * g:32 * g + H], in_=identity[:H, :H]
                )
        G = consts.tile([P, H], F32)   # gather: G[32g+h, h] = 1
        G3 = consts.tile([P, H], F32)
        for src, dst in ((G2, G), (G3s, G3)):
            pt = psum_mi.tile([P, H], F32, tag="mi")
            nc.tensor.transpose(pt[:, :], src[:, :], identity[:H, :H])
            nc.vector.tensor_copy(out=dst[:, :], in_=pt[:, :])

        # qb broadcast to the col-tiled partition layout: qb128[32g+h] = qb[h]
        qb128 = consts.tile([P, bpc], F32)
        for i in range(bpc):
            pt = psum_mi.tile([P, 1], F32, tag="mi")
            nc.tensor.matmul(pt[:, :], G2[:, :], qb[:, i:i + 1], start=True, stop=True)
            nc.vector.tensor_copy(out=qb128[:, i:i + 1], in_=pt[:, :])

        wk0 = wkT32[:, 0, :]
        wk1 = wkT32[:, 1, :]

        # --- main loop ------------------------------------------------------
        batch_tiles = []
        for i in range(bpc):
            prob = probp.tile([P, PACKED], F32, tag="prob")
            prob8 = probp.tile([H, REM_ROWS], F32, tag="prob8")
            sums = small.tile([P, NPACKS], F32, tag="sums")
            srem = small.tile([H, 1], F32, tag="srem")
            ncopy = 0
            last_k0 = None

            def chunk_scores(scp, ld, c, nsub, sub0):
                """transpose+copy+matmul chunk c (blocks s=sub0..sub0+nsub)
                into scp[32c:32c+8, :]."""
                nonlocal ncopy, last_k0
                w = nsub * P
                # both d-halves in ONE psum tile = exactly one 2KB bank:
                # 4 bufs fit -> 4 chunks in flight, and one copy per chunk
                kt = psum_kt.tile([P, 2, CHUNK], ld_dtype, tag="kt")
                for s in range(nsub):
                    nc.tensor.transpose(
                        kt[:, 0, s * P:(s + 1) * P], ld[:, sub0 + s, 0:P],
                        id_t[:, :]
                    )
                    nc.tensor.transpose(
                        kt[:, 1, s * P:(s + 1) * P], ld[:, sub0 + s, P:2 * P],
                        id_t[:, :]
                    )
                k01 = kts.tile([P, 2, CHUNK], BF16, tag="k01")
                last_k0 = k01
                if ncopy % 2 == 0:
                    nc.vector.tensor_copy(out=k01[:, :, :w], in_=kt[:, :, :w])
                else:
                    nc.scalar.copy(out=k01[:, :, :w], in_=kt[:, :, :w])
                ncopy += 1
                nc.tensor.matmul(
                    scp[32 * c:32 * c + 32, :w], wk0, k01[:, 0, :w],
                    start=True, stop=False, tile_position=(0, 32 * c),
                )
                nc.tensor.matmul(
                    scp[32 * c:32 * c + 32, :w], wk1, k01[:, 1, :w],
                    start=False, stop=True, tile_position=(0, 32 * c),
                )

            # 32-row remainder: load + process FIRST, off the critical tail
            rem_ld = loads.tile([REM_ROWS, D], ld_dtype, tag="rem_ld")
            load_dma(rem_ld[:, :], k_in[i, REM_N0:REM_N0 + REM_ROWS, :])
            scp = psum_sc.tile([P, CHUNK], F32, tag="sc")
            kt = psum_kt.tile([P, 2, CHUNK], ld_dtype, tag="kt")
            nc.tensor.transpose(
                kt[:, 0, :REM_ROWS], rem_ld[:, 0:P], id_t[:REM_ROWS, :REM_ROWS]
            )
            nc.tensor.transpose(
                kt[:, 1, :REM_ROWS], rem_ld[:, P:2 * P], id_t[:REM_ROWS, :REM_ROWS]
            )
            k01 = kts.tile([P, 2, CHUNK], BF16, tag="k01")
            nc.vector.tensor_copy(out=k01[:, :, :REM_ROWS], in_=kt[:, :, :REM_ROWS])
            nc.tensor.matmul(
                scp[:H, :REM_ROWS], wk0[:, :H], k01[:, 0, :REM_ROWS],
                start=True, stop=False,
            )
            nc.tensor.matmul(
                scp[:H, :REM_ROWS], wk1[:, :H], k01[:, 1, :REM_ROWS],
                start=False, stop=True,
            )
            nc.scalar.activation(
                out=prob8[:, :],
                in_=scp[:H, :REM_ROWS],
                func=mybir.ActivationFunctionType.Exp,
                bias=qb[:, i:i + 1],
                scale=1.0,
                accum_out=srem[:, :],
            )

            for (L, (n0, smod)) in enumerate(LOADS):
                rows = P * smod
                if i == 0 and L < len(pre_lds):
                    ld = pre_lds[L]
                else:
                    ld = loads.tile([P, 16, D], ld_dtype, tag="load")
                    # partition p <- rows [n0+smod*p, n0+smod*(p+1)): one
                    # contiguous smod-KB descriptor per partition.
                    load_dma(
                        ld[:, :smod, :],
                        k_in[i, n0:n0 + rows, :].rearrange("(p s) d -> p s d", p=P),
                    )
                scp = psum_sc.tile([P, CHUNK], F32, tag="sc")
                for c in range(smod // 4):
                    chunk_scores(scp, ld, c, 4, 4 * c)
                if smod == 12:
                    # tail pack has no group 3: write real zeros there so the
                    # exp input is deterministic and finite
                    nc.tensor.matmul(
                        scp[96:128, :], z32, last_k0[:, 0, :],
                        start=True, stop=True, tile_position=(0, 96),
                    )
                nc.scalar.activation(
                    out=prob[:, L * CHUNK:(L + 1) * CHUNK],
                    in_=scp[:, :],
                    func=mybir.ActivationFunctionType.Exp,
                    bias=qb128[:, i:i + 1],
                    scale=1.0,
                    accum_out=sums[:, L:L + 1],
                )
            # total = sum over groups of pack sums (G3 excludes the tail
            # pack's unused group 3) + remainder
            rmain = small.tile([P, 1], F32, tag="rmain")
            nc.vector.reduce_sum(
                out=rmain[:, :], in_=sums[:, :NPACKS - 1], axis=mybir.AxisListType.X
            )
            pt8 = psum_mi.tile([H, 1], F32, tag="mi")
            nc.tensor.matmul(pt8[:, :], G[:, :], rmain[:, :], start=True, stop=False)
            nc.tensor.matmul(
                pt8[:, :], G3[:, :], sums[:, NPACKS - 1:NPACKS],
                start=False, stop=True,
            )
            tot = small.tile([H, 1], F32, tag="tot")
            nc.vector.tensor_add(tot[:, :], pt8[:, :], srem[:, :])
            rec = small.tile([H, 1], F32, tag="rec")
            nc.vector.reciprocal(out=rec[:, :], in_=tot[:, :])
            rec128 = small.tile([P, 1], F32, tag="rec128")
            ptr = psum_mi.tile([P, 1], F32, tag="mi")
            nc.tensor.matmul(ptr[:, :], G2[:, :], rec[:, :], start=True, stop=True)
            nc.vector.tensor_copy(out=rec128[:, :], in_=ptr[:, :])

            # scale writes a bf16 copy (the store dtype) so stores need no
            # in-DMA cast; 4 column pieces alternate DVE/ACT for latency
            prob_bf = probp.tile([P, PACKED], BF16, tag="prob_bf")
            prob8_bf = probp.tile([H, REM_ROWS], BF16, tag="prob8_bf")
            qp = PACKED // 4
            for pc in range(4):
                sl = slice(pc * qp, (pc + 1) * qp)
                if pc % 2 == 0:
                    nc.vector.tensor_scalar_mul(
                        prob_bf[:, sl], prob[:, sl], rec128[:, :]
                    )
                else:
                    nc.scalar.mul(prob_bf[:, sl], prob[:, sl], rec128[:, :])
            nc.scalar.mul(prob8_bf[:, :], prob8[:, :], rec[:, :])
            batch_tiles.append((prob_bf, prob8_bf))

        # packed stores, DEFERRED until after every load is emitted: they
        # share the GpSimd SWDGE queue with the loads (f32 -> bf16 cast),
        # so batch 0's stores must queue behind batch 1's loads to avoid
        # stalling the load stream.
        for i, (prob, prob8) in enumerate(batch_tiles):
            for g in range(4):
                width = PACKED if g < 3 else PACKED - CHUNK
                off = g * PACKED
                store_dma(
                    out[i, :, off:off + width],
                    prob[32 * g:32 * g + H, :width],
                )
            store_dma(out[i, :, RAW_COLS - REM_ROWS:], prob8[:, :])

    nc.compile()
    return nc


_NC_CACHE = {}


def _get_nc():
    if "nc" not in _NC_CACHE:
        _NC_CACHE["nc"] = build_kernel()
    return _NC_CACHE["nc"]


def unpermute(raw):
    """raw [B, H, RAW_COLS] packed bf16 layout -> [B, H, N] f32 natural."""
    return np.ascontiguousarray(np.asarray(raw).astype(np.float32)[:, :, POS])


def kernel(query, key, W, b):
    from concourse.bass_utils import run_bass_kernel_spmd

    query = np.ascontiguousarray(np.asarray(query, np.float32).reshape(B, D))
    key = np.ascontiguousarray(np.asarray(key, np.float32))
    W = np.ascontiguousarray(np.asarray(W, np.float32))
    b = np.ascontiguousarray(np.asarray(b, np.float32))

    nc = _get_nc()
    in_maps = []
    for c in range(NCORES):
        s = slice(BPC * c, BPC * (c + 1))
        in_maps.append(
            {
                "q": query[s],
                "k": key[s],
                "w": W,
                "b": b,
            }
        )
    res = run_bass_kernel_spmd(nc, in_maps, list(range(NCORES))).results
    raw = np.concatenate([res[c]["out"] for c in range(NCORES)], axis=0)
    return unpermute(raw)
